# revision 1
# baseline (speedup 1.0000x reference)
"""Causal attention layer (B=4, N=2048, C=1024, H=16, D=64) on 8 TRN2 NeuronCores.

Sharding: core c -> (batch b = c//2, head-group g = c%2 of 8 heads).
Per core, for its (b, g):
  qkvT  = wqkvT_g.T-contract(x_b)      q,k transposed [o,n]; v transposed then
                                       DMA-transposed to [kn, 1|v] (ones col first)
  S_T   = kT.T @ qT                    pre-transposed scores [kn, qn], exp -> P_T bf16
  oT    = [1|v].T @ P_T                row 0 = softmax denominators, rows 1:65 = out.T
  attn_outT = oT[1:65] * bcast(1/oT[0])  (recip_approx_fast + gpsimd partition_broadcast)
  out_part  = attn_outT.T-contract(projT_g)
Host sums the two head-group partials per batch and adds proj_b.

The emission stream is software-pipelined: only the first v o-tile and pair-0
q/k run up front; all later v o-tiles, v_ext transposes, q/k projections, and
the output projection are queued as fillers and interleaved into the attention
stream so the PE stays dense (HAM-warm) while ScalarE runs exp.
"""
import sys

sys.path.insert(0, "/opt/trn_rl_repo")

import numpy as np

import concourse.bass as bass  # noqa: F401
import concourse.tile as tile
from concourse import bacc, mybir
from concourse.bass_utils import run_bass_kernel_spmd

F32 = mybir.dt.float32
F32R = mybir.dt.float32r
BF16 = mybir.dt.bfloat16
EXP = mybir.ActivationFunctionType.Exp

B, N, C, H, D = 4, 2048, 1024, 16, 64
G = 8            # heads per core
GC = G * D       # 512 channels per core
NT = N // 128    # 16 row tiles
NS = N // 512    # 4 row supers
CK = C // 128    # 8 contraction chunks

_cache = {}


def _build_nc():
    from contextlib import ExitStack

    nc = bacc.Bacc("TRN2", target_bir_lowering=False, debug=False)

    xT_d = nc.dram_tensor("xT", [C, N], F32R, kind="ExternalInput")
    wqkvT_d = nc.dram_tensor("wqkvT", [C, 3 * GC], F32R, kind="ExternalInput")
    projT_d = nc.dram_tensor("projT", [GC, C], BF16, kind="ExternalInput")
    tril_d = nc.dram_tensor("tril", [128, 128], BF16, kind="ExternalInput")
    ident_d = nc.dram_tensor("ident", [128, 128], BF16, kind="ExternalInput")
    onesb_d = nc.dram_tensor("onesb", [128, NT], BF16, kind="ExternalInput")
    out_d = nc.dram_tensor("out", [N, C], F32, kind="ExternalOutput")

    with tile.TileContext(nc) as tc:
        with ExitStack() as ctx:
            consts = ctx.enter_context(tc.tile_pool(name="consts", bufs=1))
            qk_pool = ctx.enter_context(tc.tile_pool(name="qk", bufs=4))
            vext_pool = ctx.enter_context(tc.tile_pool(name="vext", bufs=1))
            vT_pool = ctx.enter_context(tc.tile_pool(name="vT", bufs=2))
            w_pool = ctx.enter_context(tc.tile_pool(name="wA", bufs=3))
            xT_pool = ctx.enter_context(tc.tile_pool(name="xT", bufs=1))
            rf_pool = ctx.enter_context(tc.tile_pool(name="rf", bufs=2))
            bcs_pool = ctx.enter_context(tc.tile_pool(name="bcs", bufs=2))
            tmp_pool = ctx.enter_context(tc.tile_pool(name="tmp", bufs=2))
            ob_pool = ctx.enter_context(tc.tile_pool(name="ob", bufs=2))
            pj_pool = ctx.enter_context(tc.tile_pool(name="pj", bufs=1))
            psA = ctx.enter_context(tc.tile_pool(name="psA", bufs=2, space="PSUM"))

            tril_sb = consts.tile([128, 128], BF16)
            nc.sync.dma_start(tril_sb[:], tril_d[:])
            ident_sb = consts.tile([128, 128], BF16)
            nc.sync.dma_start(ident_sb[:], ident_d[:])

            early_w = {}

            def load_w(ot):
                if ot in early_w:
                    return early_w.pop(ot)
                wt = w_pool.tile([128, CK, 128], F32R, tag="wt", name=f"wt{ot}")
                src = wqkvT_d[:, 128 * ot:128 * (ot + 1)].rearrange(
                    "(cc p) o -> p cc o", p=128
                )
                nc.sync.dma_start(wt[:], src)
                return wt

            early_w.update({8: load_w(8), 0: load_w(0), 4: load_w(4)})

            v_ext = [vext_pool.tile([128, NT * 128], BF16, tag=f"ve{h}", name=f"ve{h}")
                     for h in range(G)]
            ve3 = [v.rearrange("p (n e) -> p n e", e=128) for v in v_ext]
            for h in range(G):
                nc.sync.dma_start(ve3[h][:, :, 0:1], onesb_d[:, :, None])

            xs = []
            for cc in range(CK):
                t = xT_pool.tile([128, N], F32R, tag=f"x{cc}", name=f"x{cc}")
                nc.sync.dma_start(t[:], xT_d[128 * cc:128 * (cc + 1), :])
                xs.append(t)

            pj_sb = [pj_pool.tile([128, C], BF16, tag=f"pj{i}", name=f"pj{i}")
                     for i in range(4)]
            for ac in range(4):
                nc.sync.dma_start(pj_sb[ac][:], projT_d[128 * ac:128 * (ac + 1), :])

            def qkv_quarter(wt, sup):
                psq = psA.tile([128, 512], F32, tag="qa", name="psq")
                for cc in range(CK):
                    nc.tensor.matmul(
                        psq[:],
                        wt[:, cc, :],
                        xs[cc][:, 512 * sup:512 * (sup + 1)],
                        start=(cc == 0),
                        stop=(cc == CK - 1),
                    )
                return psq

            tr_insts = [[None] * NT for _ in range(G)]

            # ------- step builders (emitted inline or queued as fillers) --------
            def v_steps(vp):
                """v o-tile vp -> vT (bf16) -> DMA-transpose into v_ext[2vp,2vp+1]."""
                vt = vT_pool.tile([128, N], BF16, tag="vt", name=f"vt{vp}")
                cps = [None] * NS
                steps = [("w", lambda vp=vp: load_w(8 + vp))]
                for sup in range(NS):
                    def _mms(wt, vt=vt, sup=sup):
                        psq = qkv_quarter(wt, sup)
                        cps[sup] = nc.vector.tensor_copy(
                            vt[:, 512 * sup:512 * (sup + 1)], psq[:]
                        )
                    steps.append(("q", _mms))
                for nt in range(NT):
                    def _tr(vt=vt, vp=vp, nt=nt):
                        tp = psA.tile([128, 128], BF16, tag="qa", name="tp")
                        nc.tensor.transpose(
                            tp[:], vt[:, 128 * nt:128 * (nt + 1)], ident_sb[:]
                        )
                        nc.vector.tensor_copy(
                            ve3[2 * vp][:, nt, 64:128], tp[:, 0:64]
                        )
                        nc.vector.tensor_copy(
                            ve3[2 * vp + 1][:, nt, 64:128], tp[:, 64:128]
                        )
                    steps.append(("p", _tr))
                return steps

            def qk_steps(p, interleave=False):
                qT = qk_pool.tile([128, N], F32R, tag="qk", name=f"q{p}")
                kT = qk_pool.tile([128, N], F32R, tag="qk", name=f"k{p}")
                if not interleave:
                    steps = []
                    for dst, ot in ((qT, p), (kT, 4 + p)):
                        steps.append(("w", lambda ot=ot: load_w(ot)))
                        for sup in range(NS):
                            def _mms(wt, dst=dst, sup=sup):
                                psq = qkv_quarter(wt, sup)
                                nc.vector.tensor_copy(
                                    dst[:, 512 * sup:512 * (sup + 1)], psq[:]
                                )
                            steps.append(("q", _mms))
                    return qT, kT, steps
                wts = {}
                steps = [
                    ("w", lambda: wts.setdefault("q", load_w(p))),
                    ("w", lambda: wts.setdefault("k", load_w(4 + p))),
                ]
                for sup in range(NS):
                    for key, dst in (("q", qT), ("k", kT)):
                        def _mms(_wt, key=key, dst=dst, sup=sup):
                            psq = qkv_quarter(wts[key], sup)
                            nc.vector.tensor_copy(
                                dst[:, 512 * sup:512 * (sup + 1)], psq[:]
                            )
                        steps.append(("q", _mms))
                return qT, kT, steps

            attn_outT = None

            def proj_steps(s):
                steps = []
                for nt in range(4 * s, 4 * s + 4):
                    for oc in (0, 1):
                        def _pj(nt=nt, oc=oc):
                            pp = psA.tile([128, 512], F32, tag="qa", name="pp")
                            for ac in range(4):
                                nc.tensor.matmul(
                                    pp[:],
                                    attn_outT[ac][:, 128 * nt:128 * (nt + 1)],
                                    pj_sb[ac][:, 512 * oc:512 * (oc + 1)],
                                    start=(ac == 0),
                                    stop=(ac == 3),
                                )
                            ob = ob_pool.tile([128, 512], F32, tag="ob", name="ob")
                            nc.vector.tensor_copy(ob[:], pp[:])
                            nc.sync.dma_start(
                                out_d[128 * nt:128 * (nt + 1),
                                      512 * oc:512 * (oc + 1)],
                                ob[:],
                            )
                        steps.append(("p", _pj))
                return steps

            # filler machinery: fill(n) emits until n PE-carrying steps are out
            pending = []
            state = {"wt": None}

            def fill(n):
                done = 0
                while pending and done < n:
                    kind, fn = pending.pop(0)
                    if kind == "w":
                        state["wt"] = fn()
                    elif kind == "q":
                        fn(state["wt"])
                        done += 1
                    elif kind == "t":
                        fn()
                    else:
                        fn()
                        done += 1

            def fill_all():
                while pending:
                    fill(4)

            # ---------------- prologue: vp0 + pair-0 q/k ------------------------
            for kind, fn in v_steps(0):
                if kind == "w":
                    state["wt"] = fn()
                elif kind == "q":
                    fn(state["wt"])
                else:
                    fn()
            qT, kT, steps0 = qk_steps(0, interleave=True)
            pending.extend(steps0)

            # ---------------- attention pair loop (with fillers) ----------------
            with (
                tc.tile_pool(name="aoT", bufs=1) as aoT_pool,
                tc.tile_pool(name="pt", bufs=7) as pt_pool,
                tc.tile_pool(name="psS", bufs=2, space="PSUM") as psS,
                tc.tile_pool(name="psO", bufs=2, space="PSUM") as psO,
            ):
                attn_outT = [aoT_pool.tile([128, N], BF16, tag=f"ao{p}", name=f"ao{p}")
                             for p in range(4)]
                for p in range(4):
                    if p < 3:
                        pending.extend(v_steps(p + 1))
                        nq, nk_, nsteps = qk_steps(p + 1)
                        pending.extend(nsteps)

                    for s in range(NS):
                        if p == 0:
                            fill(2)
                        nkb = 4 * (s + 1)
                        pts = {0: [], 1: []}
                        for kg in range(nkb // 2):
                            for h in (0, 1):
                                fill(2)
                                hh = slice(64 * h, 64 * (h + 1))
                                S2 = psS.tile([128, 1024], F32, tag="s2", name="S2")
                                for j in (0, 1):
                                    k = 2 * kg + j
                                    nc.tensor.matmul(
                                        S2[:, 512 * j:512 * (j + 1)],
                                        kT[hh, 128 * k:128 * (k + 1)],
                                        qT[hh, 512 * s:512 * (s + 1)],
                                    )
                                P2 = pt_pool.tile([128, 1024], BF16, tag="pt", name="P2")
                                nc.scalar.activation(
                                    P2[:], S2[:], EXP, scale=float(D) ** -0.5
                                )
                                for j in (0, 1):
                                    k = 2 * kg + j
                                    if k >= 4 * s:
                                        ridx = k - 4 * s
                                        c0 = 512 * j
                                        if ridx > 0:
                                            nc.vector.memset(
                                                P2[:, c0:c0 + 128 * ridx], 0.0
                                            )
                                        nc.vector.tensor_mul(
                                            P2[:, c0 + 128 * ridx:c0 + 128 * (ridx + 1)],
                                            P2[:, c0 + 128 * ridx:c0 + 128 * (ridx + 1)],
                                            tril_sb[:],
                                        )
                                pts[h].append(P2)
                        for h in (0, 1):
                            hg = 2 * p + h
                            oT = psO.tile([128, 512], F32, tag="oT", name="oT")
                            for kg in range(nkb // 2):
                                fill(2)
                                for j in (0, 1):
                                    k = 2 * kg + j
                                    nc.tensor.matmul(
                                        oT[:],
                                        ve3[hg][:, k, :],
                                        pts[h][kg][:, 512 * j:512 * (j + 1)],
                                        start=(k == 0),
                                        stop=(k == nkb - 1),
                                    )
                            Rf = rf_pool.tile([1, 512], F32, tag="rf", name="Rf")
                            nc.vector.reciprocal_approx_fast(Rf[:], oT[0:1, :])
                            bcs = bcs_pool.tile([128, 512], F32, tag="bcs", name="bcs")
                            nc.gpsimd.partition_broadcast(bcs[:], Rf[:])
                            tmp = tmp_pool.tile([128, 512], BF16, tag="tmp", name="tmp")
                            nc.vector.tensor_mul(tmp[:], oT[:], bcs[:])
                            nc.sync.dma_start(
                                attn_outT[p][64 * h:64 * (h + 1),
                                             512 * s:512 * (s + 1)],
                                tmp[64:128, :],
                            )
                        if p == 3:
                            pending.extend(proj_steps(s))
                            fill(2)
                    if p < 3:
                        qT, kT = nq, nk_
                fill_all()

    nc.compile()
    return nc


def _tril_np():
    import ml_dtypes

    i = np.arange(128)[:, None]
    j = np.arange(128)[None, :]
    return (j >= i).astype(np.float32).astype(ml_dtypes.bfloat16)


def make_in_maps(x, qkv_w, proj_w):
    x = np.asarray(x, dtype=np.float32)
    qkv_w = np.asarray(qkv_w, dtype=np.float32)
    proj_w = np.asarray(proj_w, dtype=np.float32)
    tril = _tril_np()
    in_maps = []
    for c in range(8):
        b, g = c // 2, c % 2
        sl = slice(g * GC, (g + 1) * GC)
        wq, wk, wv = qkv_w[0:C][sl], qkv_w[C:2 * C][sl], qkv_w[2 * C:3 * C][sl]
        in_maps.append(
            {
                "xT": np.ascontiguousarray(x[b].T),
                "wqkvT": np.ascontiguousarray(np.concatenate([wq, wk, wv], 0).T),
                "projT": np.ascontiguousarray(proj_w[:, sl].T).astype(
                    __import__("ml_dtypes").bfloat16
                ),
                "tril": tril,
                "ident": np.eye(128, dtype=np.float32).astype(
                    __import__("ml_dtypes").bfloat16
                ),
                "onesb": np.ones((128, NT), dtype=np.float32).astype(
                    __import__("ml_dtypes").bfloat16
                ),
            }
        )
    return in_maps


def kernel(x, qkv_w, proj_w, proj_b):
    proj_b = np.asarray(proj_b, dtype=np.float32)

    if "nc" not in _cache:
        _cache["nc"] = _build_nc()
    nc = _cache["nc"]

    in_maps = make_in_maps(x, qkv_w, proj_w)
    res = run_bass_kernel_spmd(nc, in_maps, core_ids=list(range(8)))
    out = np.stack(
        [res.results[2 * b]["out"] + res.results[2 * b + 1]["out"] for b in range(B)], 0
    )
    return (out + proj_b[None, None, :]).astype(np.float32)



# revision 2
# speedup vs baseline: 1.2601x; 1.2601x over previous
"""Causal attention layer (B=4, N=2048, C=1024, H=16, D=64) on 8 TRN2 NeuronCores.

Sharding: core c -> (batch b = c//2, head-group g = c%2 of 8 heads).
All matmul operands are bf16 (fp32 lowers to two-pass fp32_mode=HIGH matmuls at
~1.5x the cost and defeats fast-weight-load).

Per core, for its (b, g), heads are processed as 4 pairs p (heads 2p, 2p+1):
  qT/kT[pair]  = wT_pair.T-contract(x)          [128 chan, N tok] bf16
  v            = x.T-contract(wv)               token-major [128 tok, 512 chan]
                 copied straight into ve_all[key, nt, head, 64:128] (no PE
                 transposes; col 0 of each 128-block holds ones for denominators)
  S2[k-tile]   = [kT[h0].T@qT[h0] | kT[h1].T@qT[h1]]   two K=64 matmuls on
                 disjoint PE row groups (tile_position (0,0)/(64,0)) -> overlap
  P2           = exp(S2 * D^-0.5) bf16, causal-masked on diagonal tiles
  oT[:,512h]  += ve_all[:,k,2p+h,:].T @ P2[:,512h]   row0 = denom, rows 64:128 = out
  attn_outT    = oT[64:128] * bcast(1/oT[0])    (recip + gpsimd partition_broadcast)
  out_part     = attn_outT.T-contract(projT)
Host sums the two head-group partials per batch and adds proj_b.

QKV chains for pair p+1, v chains, and the output projection are queued as
fillers and paced into the attention stream so the PE stays dense while ScalarE
runs exp.
"""
import sys

sys.path.insert(0, "/opt/trn_rl_repo")

import numpy as np

import concourse.bass as bass  # noqa: F401
import concourse.tile as tile
from concourse import bacc, mybir
from concourse.bass_utils import run_bass_kernel_spmd

F32 = mybir.dt.float32
BF16 = mybir.dt.bfloat16
EXP = mybir.ActivationFunctionType.Exp

B, N, C, H, D = 4, 2048, 1024, 16, 64
G = 8            # heads per core
GC = G * D       # 512 channels per core
NT = N // 128    # 16 row tiles
NS = N // 512    # 4 row supers
CK = C // 128    # 8 contraction chunks

_cache = {}


def _build_nc():
    from contextlib import ExitStack

    nc = bacc.Bacc("TRN2", target_bir_lowering=False, debug=False)

    xT_d = nc.dram_tensor("xT", [C, N], BF16, kind="ExternalInput")
    wqkvT_d = nc.dram_tensor("wqkvT", [C, 3 * GC], BF16, kind="ExternalInput")
    projT_d = nc.dram_tensor("projT", [GC, C], BF16, kind="ExternalInput")
    tril_d = nc.dram_tensor("tril", [128, 128], BF16, kind="ExternalInput")
    out_d = nc.dram_tensor("out", [N, C], F32, kind="ExternalOutput")

    with tile.TileContext(nc) as tc:
        with ExitStack() as ctx:
            consts = ctx.enter_context(tc.tile_pool(name="consts", bufs=1))
            qk_pool = ctx.enter_context(tc.tile_pool(name="qk", bufs=4))
            ve_pool = ctx.enter_context(tc.tile_pool(name="ve", bufs=1))
            wv_pool = ctx.enter_context(tc.tile_pool(name="wv", bufs=1))
            w_pool = ctx.enter_context(tc.tile_pool(name="wA", bufs=4))
            xT_pool = ctx.enter_context(tc.tile_pool(name="xT", bufs=1))
            rf_pool = ctx.enter_context(tc.tile_pool(name="rf", bufs=2))
            bcs_pool = ctx.enter_context(tc.tile_pool(name="bcs", bufs=2))
            tmp_pool = ctx.enter_context(tc.tile_pool(name="tmp", bufs=2))
            oTs_pool = ctx.enter_context(tc.tile_pool(name="oTs", bufs=2))
            ob_pool = ctx.enter_context(tc.tile_pool(name="ob", bufs=2))
            pj_pool = ctx.enter_context(tc.tile_pool(name="pj", bufs=1))
            aoT_pool = ctx.enter_context(tc.tile_pool(name="aoT", bufs=1))
            pt_pool = ctx.enter_context(tc.tile_pool(name="pt", bufs=4))
            psq = ctx.enter_context(tc.tile_pool(name="psq", bufs=2, space="PSUM"))
            psS = ctx.enter_context(tc.tile_pool(name="psS", bufs=2, space="PSUM"))
            psO = ctx.enter_context(tc.tile_pool(name="psO", bufs=1, space="PSUM"))

            tril_sb = consts.tile([128, 128], BF16)
            nc.sync.dma_start(tril_sb[:], tril_d[:])

            wv_sb = wv_pool.tile([128, CK, GC], BF16, name="wv")
            nc.sync.dma_start(
                wv_sb[:],
                wqkvT_d[:, 2 * GC:3 * GC].rearrange("(cc p) o -> p cc o", p=128),
            )

            # x tiles, loaded super-major so compute can start after ~1MB
            xs = [xT_pool.tile([128, N], BF16, tag=f"x{cc}", name=f"x{cc}")
                  for cc in range(CK)]
            for sup in range(NS):
                for cc in range(CK):
                    nc.sync.dma_start(
                        xs[cc][:, 512 * sup:512 * (sup + 1)],
                        xT_d[128 * cc:128 * (cc + 1), 512 * sup:512 * (sup + 1)],
                    )

            def load_w(ot):
                wt = w_pool.tile([128, CK, 128], BF16, tag="wt", name=f"wt{ot}")
                src = wqkvT_d[:, 128 * ot:128 * (ot + 1)].rearrange(
                    "(cc p) o -> p cc o", p=128
                )
                nc.sync.dma_start(wt[:], src)
                return wt

            pj_sb = [pj_pool.tile([128, C], BF16, tag=f"pj{i}", name=f"pj{i}")
                     for i in range(4)]
            for ac in range(4):
                nc.sync.dma_start(pj_sb[ac][:], projT_d[128 * ac:128 * (ac + 1), :])

            # ve_all[key, nt, head, col]: col 0 = ones, cols 64:128 = v
            ve_all = ve_pool.tile([128, NT, G, 128], BF16, name="ve")
            nc.vector.memset(ve_all[:, :, :, 0:1], 1.0)

            def v_chain(nt):
                psv = psq.tile([128, GC], F32, tag="qa", name=f"psv{nt}")
                for cc in range(CK):
                    nc.tensor.matmul(
                        psv[:],
                        xs[cc][:, 128 * nt:128 * (nt + 1)],
                        wv_sb[:, cc, :],
                        start=(cc == 0),
                        stop=(cc == CK - 1),
                    )
                nc.vector.tensor_copy(
                    ve_all[:, nt, :, 64:128],
                    psv[:].rearrange("p (h d) -> p h d", h=G),
                )

            def qk_chain(wt, dst, sup):
                pq = psq.tile([128, 512], F32, tag="qa", name="pq")
                for cc in range(CK):
                    nc.tensor.matmul(
                        pq[:],
                        wt[:, cc, :],
                        xs[cc][:, 512 * sup:512 * (sup + 1)],
                        start=(cc == 0),
                        stop=(cc == CK - 1),
                    )
                nc.vector.tensor_copy(dst[:, 512 * sup:512 * (sup + 1)], pq[:])

            attn_outT = [aoT_pool.tile([128, N], BF16, tag=f"ao{p}", name=f"ao{p}")
                         for p in range(4)]

            def proj_step(nt):
                for oc in (0, 1):
                    pp = psq.tile([128, 512], F32, tag="qa", name="pp")
                    for ac in range(4):
                        nc.tensor.matmul(
                            pp[:],
                            attn_outT[ac][:, 128 * nt:128 * (nt + 1)],
                            pj_sb[ac][:, 512 * oc:512 * (oc + 1)],
                            start=(ac == 0),
                            stop=(ac == 3),
                        )
                    ob = ob_pool.tile([128, 512], F32, tag="ob", name="ob")
                    nc.vector.tensor_copy(ob[:], pp[:])
                    nc.sync.dma_start(
                        out_d[128 * nt:128 * (nt + 1), 512 * oc:512 * (oc + 1)],
                        ob[:],
                    )

            # ---------------- filler machinery ----------------
            pending = []
            state = {}

            def fill(n):
                done = 0
                while pending and done < n:
                    kind, fn = pending.pop(0)
                    if kind == "w":
                        fn()
                    else:
                        fn()
                        done += 1

            def fill_all():
                while pending:
                    fill(4)

            def queue_pair(p):
                """Queue q/k chains for pair p (weights + 8 chains)."""
                qT = qk_pool.tile([128, N], BF16, tag="qk", name=f"q{p}")
                kT = qk_pool.tile([128, N], BF16, tag="qk", name=f"k{p}")

                def _wq():
                    state[f"wq{p}"] = load_w(p)

                def _wk():
                    state[f"wk{p}"] = load_w(4 + p)

                steps = [("w", _wq), ("w", _wk)]
                for sup in range(NS):
                    for key, dst in ((f"wq{p}", qT), (f"wk{p}", kT)):
                        def _c(key=key, dst=dst, sup=sup):
                            qk_chain(state[key], dst, sup)
                        steps.append(("q", _c))
                return qT, kT, steps

            # ---------------- prologue ----------------
            for nt in range(4):
                v_chain(nt)
            wq0, wk0 = load_w(0), load_w(4)
            qT = qk_pool.tile([128, N], BF16, tag="qk", name="q0")
            kT = qk_pool.tile([128, N], BF16, tag="qk", name="k0")
            qk_chain(wq0, qT, 0)
            qk_chain(wk0, kT, 0)
            for sup in range(1, NS):
                for wt, dst in ((wq0, qT), (wk0, kT)):
                    def _c(wt=wt, dst=dst, sup=sup):
                        qk_chain(wt, dst, sup)
                    pending.append(("q", _c))

            # ---------------- attention pair loop ----------------
            for p in range(4):
                if p < 3:
                    nq, nk, nsteps = queue_pair(p + 1)
                    pending.extend(nsteps)

                for s in range(NS):
                    if p == 0 and s > 0:
                        for nt in range(4 * s, 4 * s + 4):
                            v_chain(nt)
                    nkb = 4 * (s + 1)
                    oT = psO.tile([128, 1024], F32, tag="oT", name="oT")
                    P_prev = None
                    for k in range(nkb):
                        if pending and (k % (5 if p < 3 else 2) == 0):
                            fill(1)
                        S2 = psS.tile([128, 1024], F32, tag="s2", name="S2")
                        for h in (0, 1):
                            hh = slice(64 * h, 64 * (h + 1))
                            nc.tensor.matmul(
                                S2[:, 512 * h:512 * (h + 1)],
                                kT[hh, 128 * k:128 * (k + 1)],
                                qT[hh, 512 * s:512 * (s + 1)],
                            )
                        P2 = pt_pool.tile([128, 1024], BF16, tag="pt", name="P2")
                        nc.scalar.activation(
                            P2[:], S2[:], EXP, scale=float(D) ** -0.5
                        )
                        r = k - 4 * s
                        if r >= 0:
                            for h in (0, 1):
                                c0 = 512 * h
                                if r > 0:
                                    nc.vector.memset(P2[:, c0:c0 + 128 * r], 0.0)
                                nc.vector.tensor_mul(
                                    P2[:, c0 + 128 * r:c0 + 128 * (r + 1)],
                                    P2[:, c0 + 128 * r:c0 + 128 * (r + 1)],
                                    tril_sb[:],
                                )
                        if P_prev is not None:
                            kp = k - 1
                            for h in (0, 1):
                                nc.tensor.matmul(
                                    oT[:, 512 * h:512 * (h + 1)],
                                    ve_all[:, kp, 2 * p + h, :],
                                    P_prev[:, 512 * h:512 * (h + 1)],
                                    start=(kp == 0),
                                    stop=(kp == nkb - 1),
                                )
                        P_prev = P2
                    kp = nkb - 1
                    for h in (0, 1):
                        nc.tensor.matmul(
                            oT[:, 512 * h:512 * (h + 1)],
                            ve_all[:, kp, 2 * p + h, :],
                            P_prev[:, 512 * h:512 * (h + 1)],
                            start=(kp == 0),
                            stop=(kp == nkb - 1),
                        )
                    # evacuate psum fast, then normalize from SBUF
                    oTs = oTs_pool.tile([128, 1024], F32, tag="oTs", name="oTs")
                    nc.vector.tensor_copy(oTs[:], oT[:])
                    Rf = rf_pool.tile([1, 1024], F32, tag="rf", name="Rf")
                    nc.vector.reciprocal_approx_fast(Rf[:], oTs[0:1, :])
                    bcs = bcs_pool.tile([128, 1024], F32, tag="bcs", name="bcs")
                    nc.gpsimd.partition_broadcast(bcs[:], Rf[:])
                    tmp = tmp_pool.tile([128, 1024], BF16, tag="tmp", name="tmp")
                    nc.vector.tensor_mul(
                        tmp[64:128, :], oTs[64:128, :], bcs[64:128, :]
                    )
                    for h in (0, 1):
                        nc.sync.dma_start(
                            attn_outT[p][64 * h:64 * (h + 1),
                                         512 * s:512 * (s + 1)],
                            tmp[64:128, 512 * h:512 * (h + 1)],
                        )
                    if p == 3:
                        for nt in range(4 * s, 4 * s + 4):
                            def _pj(nt=nt):
                                proj_step(nt)
                            pending.append(("p", _pj))
                if p < 3:
                    qT, kT = nq, nk
            fill_all()

    nc.compile()
    return nc


def _tril_np():
    import ml_dtypes

    i = np.arange(128)[:, None]
    j = np.arange(128)[None, :]
    return (j >= i).astype(np.float32).astype(ml_dtypes.bfloat16)


def make_in_maps(x, qkv_w, proj_w):
    import ml_dtypes

    bf16 = ml_dtypes.bfloat16
    x = np.asarray(x, dtype=np.float32)
    qkv_w = np.asarray(qkv_w, dtype=np.float32)
    proj_w = np.asarray(proj_w, dtype=np.float32)
    tril = _tril_np()
    in_maps = []
    for c in range(8):
        b, g = c // 2, c % 2
        sl = slice(g * GC, (g + 1) * GC)
        wq, wk, wv = qkv_w[0:C][sl], qkv_w[C:2 * C][sl], qkv_w[2 * C:3 * C][sl]
        in_maps.append(
            {
                "xT": np.ascontiguousarray(x[b].T).astype(bf16),
                "wqkvT": np.ascontiguousarray(
                    np.concatenate([wq, wk, wv], 0).T
                ).astype(bf16),
                "projT": np.ascontiguousarray(proj_w[:, sl].T).astype(bf16),
                "tril": tril,
            }
        )
    return in_maps


def kernel(x, qkv_w, proj_w, proj_b):
    proj_b = np.asarray(proj_b, dtype=np.float32)

    if "nc" not in _cache:
        _cache["nc"] = _build_nc()
    nc = _cache["nc"]

    in_maps = make_in_maps(x, qkv_w, proj_w)
    res = run_bass_kernel_spmd(nc, in_maps, core_ids=list(range(8)))
    out = np.stack(
        [res.results[2 * b]["out"] + res.results[2 * b + 1]["out"] for b in range(B)], 0
    )
    return (out + proj_b[None, None, :]).astype(np.float32)


# revision 5
# speedup vs baseline: 1.4197x; 1.1266x over previous
"""Causal attention layer (B=4, N=2048, C=1024, H=16, D=64) on 8 TRN2 NeuronCores.

Sharding: core c -> (batch b = c//2, head-group g = c%2 of 8 heads).
All matmul operands are bf16 (fp32 lowers to two-pass fp32_mode=HIGH matmuls at
~1.5x the cost and defeats fast-weight-load).

Per core, for its (b, g), heads are processed as 4 pairs p (heads 2p, 2p+1):
  qT/kT[pair]  = wT_pair.T-contract(x)          [128 chan, N tok] bf16
  v            = x.T-contract(wv)               token-major [128 tok, 512 chan]
                 copied straight into ve_all[key, nt, head, 64:128] (no PE
                 transposes; col 0 of each 128-block holds ones for denominators)
  S2[k-tile]   = [kT[h0].T@qT[h0] | kT[h1].T@qT[h1]]   two K=64 matmuls on
                 disjoint PE row groups (tile_position (0,0)/(64,0)) -> overlap
  P2           = exp(S2 * D^-0.5) bf16, causal-masked on diagonal tiles
  oT[:,512h]  += ve_all[:,k,2p+h,:].T @ P2[:,512h]   row0 = denom, rows 64:128 = out
  attn_outT    = oT[64:128] * bcast(1/oT[0])    (recip + gpsimd partition_broadcast)
  out_part     = attn_outT.T-contract(projT)
Host sums the two head-group partials per batch and adds proj_b.

QKV chains for pair p+1, v chains, and the output projection are queued as
fillers and paced into the attention stream so the PE stays dense while ScalarE
runs exp.
"""
import sys

sys.path.insert(0, "/opt/trn_rl_repo")

import numpy as np

import concourse.bass as bass  # noqa: F401
import concourse.tile as tile
from concourse import bacc, mybir
from concourse.bass_utils import run_bass_kernel_spmd

F32 = mybir.dt.float32
BF16 = mybir.dt.bfloat16
EXP = mybir.ActivationFunctionType.Exp

B, N, C, H, D = 4, 2048, 1024, 16, 64
G = 8            # heads per core
GC = G * D       # 512 channels per core
NT = N // 128    # 16 row tiles
NS = N // 512    # 4 row supers
CK = C // 128    # 8 contraction chunks

_cache = {}


def _build_nc():
    from contextlib import ExitStack

    nc = bacc.Bacc("TRN2", target_bir_lowering=False, debug=False)

    xT_d = nc.dram_tensor("xT", [C, N], BF16, kind="ExternalInput")
    wqkvT_d = nc.dram_tensor("wqkvT", [C, 3 * GC], BF16, kind="ExternalInput")
    projT_d = nc.dram_tensor("projT", [GC, C], BF16, kind="ExternalInput")
    tril_d = nc.dram_tensor("tril", [128, 128], BF16, kind="ExternalInput")
    out_d = nc.dram_tensor("out", [N, C], F32, kind="ExternalOutput")

    with tile.TileContext(nc) as tc:
        with ExitStack() as ctx:
            consts = ctx.enter_context(tc.tile_pool(name="consts", bufs=1))
            qk_pool = ctx.enter_context(tc.tile_pool(name="qk", bufs=4))
            ve_pool = ctx.enter_context(tc.tile_pool(name="ve", bufs=1))
            wv_pool = ctx.enter_context(tc.tile_pool(name="wv", bufs=1))
            w_pool = ctx.enter_context(tc.tile_pool(name="wA", bufs=4))
            xT_pool = ctx.enter_context(tc.tile_pool(name="xT", bufs=1))
            rf_pool = ctx.enter_context(tc.tile_pool(name="rf", bufs=2))
            bcs_pool = ctx.enter_context(tc.tile_pool(name="bcs", bufs=2))
            tmp_pool = ctx.enter_context(tc.tile_pool(name="tmp", bufs=2))
            oTs_pool = ctx.enter_context(tc.tile_pool(name="oTs", bufs=2))
            ob_pool = ctx.enter_context(tc.tile_pool(name="ob", bufs=2))
            pj_pool = ctx.enter_context(tc.tile_pool(name="pj", bufs=1))
            aoT_pool = ctx.enter_context(tc.tile_pool(name="aoT", bufs=1))
            pt_pool = ctx.enter_context(tc.tile_pool(name="pt", bufs=4))
            psq = ctx.enter_context(tc.tile_pool(name="psq", bufs=2, space="PSUM"))
            psS = ctx.enter_context(tc.tile_pool(name="psS", bufs=2, space="PSUM"))
            psO = ctx.enter_context(tc.tile_pool(name="psO", bufs=1, space="PSUM"))

            tril_sb = consts.tile([128, 128], BF16)
            nc.sync.dma_start(tril_sb[:], tril_d[:])

            wv_sb = wv_pool.tile([128, CK, GC], BF16, name="wv")
            nc.sync.dma_start(
                wv_sb[:],
                wqkvT_d[:, 2 * GC:3 * GC].rearrange("(cc p) o -> p cc o", p=128),
            )

            def load_w(ot):
                wt = w_pool.tile([128, CK, 128], BF16, tag="wt", name=f"wt{ot}")
                src = wqkvT_d[:, 128 * ot:128 * (ot + 1)].rearrange(
                    "(cc p) o -> p cc o", p=128
                )
                nc.sync.dma_start(wt[:], src)
                return wt

            # x tiles loaded super-major; pair-0 weights before x sup1-3 so the
            # first q/k chains don't wait behind 3MB of x
            xs = [xT_pool.tile([128, N], BF16, tag=f"x{cc}", name=f"x{cc}")
                  for cc in range(CK)]
            for cc in range(CK):
                nc.sync.dma_start(
                    xs[cc][:, 0:512], xT_d[128 * cc:128 * (cc + 1), 0:512]
                )
            wq0, wk0 = load_w(0), load_w(4)
            for sup in range(1, NS):
                for cc in range(CK):
                    nc.sync.dma_start(
                        xs[cc][:, 512 * sup:512 * (sup + 1)],
                        xT_d[128 * cc:128 * (cc + 1), 512 * sup:512 * (sup + 1)],
                    )

            pj_sb = [pj_pool.tile([128, C], BF16, tag=f"pj{i}", name=f"pj{i}")
                     for i in range(4)]
            for ac in range(4):
                nc.sync.dma_start(pj_sb[ac][:], projT_d[128 * ac:128 * (ac + 1), :])

            # ve_all[key, nt, head, col]: col 0 = ones, cols 64:128 = v
            ve_all = ve_pool.tile([128, NT, G, 128], BF16, name="ve")
            nc.vector.memset(ve_all[:, :, :, 0:1], 1.0)

            def v_chain(nt):
                psv = psq.tile([128, GC], F32, tag="qa", name=f"psv{nt}")
                for cc in range(CK):
                    nc.tensor.matmul(
                        psv[:],
                        xs[cc][:, 128 * nt:128 * (nt + 1)],
                        wv_sb[:, cc, :],
                        start=(cc == 0),
                        stop=(cc == CK - 1),
                    )
                nc.vector.tensor_copy(
                    ve_all[:, nt, :, 64:128],
                    psv[:].rearrange("p (h d) -> p h d", h=G),
                )

            def qk_chain(wt, dst, sup):
                pq = psq.tile([128, 512], F32, tag="qa", name="pq")
                for cc in range(CK):
                    nc.tensor.matmul(
                        pq[:],
                        wt[:, cc, :],
                        xs[cc][:, 512 * sup:512 * (sup + 1)],
                        start=(cc == 0),
                        stop=(cc == CK - 1),
                    )
                nc.vector.tensor_copy(dst[:, 512 * sup:512 * (sup + 1)], pq[:])

            attn_outT = [aoT_pool.tile([128, N], BF16, tag=f"ao{p}", name=f"ao{p}")
                         for p in range(4)]

            def proj_step(nt):
                for oc in (0, 1):
                    pp = psq.tile([128, 512], F32, tag="qa", name="pp")
                    for ac in range(4):
                        nc.tensor.matmul(
                            pp[:],
                            attn_outT[ac][:, 128 * nt:128 * (nt + 1)],
                            pj_sb[ac][:, 512 * oc:512 * (oc + 1)],
                            start=(ac == 0),
                            stop=(ac == 3),
                        )
                    ob = ob_pool.tile([128, 512], F32, tag="ob", name="ob")
                    nc.vector.tensor_copy(ob[:], pp[:])
                    nc.sync.dma_start(
                        out_d[128 * nt:128 * (nt + 1), 512 * oc:512 * (oc + 1)],
                        ob[:],
                    )

            # ---------------- filler machinery ----------------
            pending = []
            state = {}

            def fill(n):
                done = 0
                while pending and done < n:
                    kind, fn = pending.pop(0)
                    if kind == "w":
                        fn()
                    else:
                        fn()
                        done += 1

            def fill_all():
                while pending:
                    fill(4)

            def queue_pair(p):
                """Queue q/k chains for pair p (weights + 8 chains)."""
                qT = qk_pool.tile([128, N], BF16, tag="qk", name=f"q{p}")
                kT = qk_pool.tile([128, N], BF16, tag="qk", name=f"k{p}")

                def _wq():
                    state[f"wq{p}"] = load_w(p)

                def _wk():
                    state[f"wk{p}"] = load_w(4 + p)

                steps = [("w", _wq), ("w", _wk)]
                for sup in range(NS):
                    for key, dst in ((f"wq{p}", qT), (f"wk{p}", kT)):
                        def _c(key=key, dst=dst, sup=sup):
                            qk_chain(state[key], dst, sup)
                        steps.append(("q", _c))
                return qT, kT, steps

            # ---------------- prologue ----------------
            for nt in range(4):
                v_chain(nt)
            qT = qk_pool.tile([128, N], BF16, tag="qk", name="q0")
            kT = qk_pool.tile([128, N], BF16, tag="qk", name="k0")
            qk_chain(wq0, qT, 0)
            qk_chain(wk0, kT, 0)
            for sup in range(1, NS):
                for wt, dst in ((wq0, qT), (wk0, kT)):
                    def _c(wt=wt, dst=dst, sup=sup):
                        qk_chain(wt, dst, sup)
                    pending.append(("q", _c))

            # ---------------- attention pair loop ----------------
            for p in range(4):
                if p < 3:
                    nq, nk, nsteps = queue_pair(p + 1)
                    pending.extend(nsteps)

                for s in range(NS):
                    if p == 0 and s > 0:
                        for nt in range(4 * s, 4 * s + 4):
                            v_chain(nt)
                    nkb = 4 * (s + 1)
                    oT = psO.tile([128, 1024], F32, tag="oT", name="oT")
                    P_prev = None
                    def pv_pair(kp, P):
                        # diagonal tiles only touch queries >= 128*r (causal)
                        q0 = 128 * max(kp - 4 * s, 0)
                        for h in (0, 1):
                            c0 = 512 * h
                            nc.tensor.matmul(
                                oT[:, c0 + q0:c0 + 512],
                                ve_all[:, kp, 2 * p + h, :],
                                P[:, c0 + q0:c0 + 512],
                                start=(kp == 0),
                                stop=(kp == nkb - 1),
                            )

                    for k in range(nkb):
                        if pending and (k % (5 if p < 3 else 2) == 0):
                            fill(1)
                        r = k - 4 * s
                        q0 = 128 * max(r, 0)
                        S2 = psS.tile([128, 1024], F32, tag="s2", name="S2")
                        for h in (0, 1):
                            hh = slice(64 * h, 64 * (h + 1))
                            nc.tensor.matmul(
                                S2[:, 512 * h + q0:512 * (h + 1)],
                                kT[hh, 128 * k:128 * (k + 1)],
                                qT[hh, 512 * s + q0:512 * (s + 1)],
                            )
                        P2 = pt_pool.tile([128, 1024], BF16, tag="pt", name="P2")
                        s3 = S2[:].rearrange("p (h q) -> p h q", h=2)
                        p3 = P2[:].rearrange("p (h q) -> p h q", h=2)
                        nc.scalar.activation(
                            p3[:, :, q0:512], s3[:, :, q0:512], EXP,
                            scale=float(D) ** -0.5,
                        )
                        if r >= 0:
                            for h in (0, 1):
                                c0 = 512 * h
                                nc.vector.tensor_mul(
                                    P2[:, c0 + q0:c0 + q0 + 128],
                                    P2[:, c0 + q0:c0 + q0 + 128],
                                    tril_sb[:],
                                )
                        if P_prev is not None:
                            pv_pair(k - 1, P_prev)
                        P_prev = P2
                    pv_pair(nkb - 1, P_prev)
                    # evacuate psum fast, then normalize from SBUF
                    oTs = oTs_pool.tile([128, 1024], F32, tag="oTs", name="oTs")
                    nc.vector.tensor_copy(oTs[:], oT[:])
                    Rf = rf_pool.tile([1, 1024], F32, tag="rf", name="Rf")
                    nc.vector.reciprocal_approx_fast(Rf[:], oTs[0:1, :])
                    bcs = bcs_pool.tile([128, 1024], F32, tag="bcs", name="bcs")
                    nc.gpsimd.partition_broadcast(bcs[:], Rf[:])
                    tmp = tmp_pool.tile([128, 1024], BF16, tag="tmp", name="tmp")
                    nc.vector.tensor_mul(
                        tmp[64:128, :], oTs[64:128, :], bcs[64:128, :]
                    )
                    for h in (0, 1):
                        nc.sync.dma_start(
                            attn_outT[p][64 * h:64 * (h + 1),
                                         512 * s:512 * (s + 1)],
                            tmp[64:128, 512 * h:512 * (h + 1)],
                        )
                    if p == 3:
                        for nt in range(4 * s, 4 * s + 4):
                            def _pj(nt=nt):
                                proj_step(nt)
                            pending.append(("p", _pj))
                if p < 3:
                    qT, kT = nq, nk
            fill_all()

    nc.compile()
    return nc


def _tril_np():
    import ml_dtypes

    i = np.arange(128)[:, None]
    j = np.arange(128)[None, :]
    return (j >= i).astype(np.float32).astype(ml_dtypes.bfloat16)


def make_in_maps(x, qkv_w, proj_w):
    import ml_dtypes

    bf16 = ml_dtypes.bfloat16
    x = np.asarray(x, dtype=np.float32)
    qkv_w = np.asarray(qkv_w, dtype=np.float32)
    proj_w = np.asarray(proj_w, dtype=np.float32)
    tril = _tril_np()
    in_maps = []
    for c in range(8):
        b, g = c // 2, c % 2
        sl = slice(g * GC, (g + 1) * GC)
        wq, wk, wv = qkv_w[0:C][sl], qkv_w[C:2 * C][sl], qkv_w[2 * C:3 * C][sl]
        in_maps.append(
            {
                "xT": np.ascontiguousarray(x[b].T).astype(bf16),
                "wqkvT": np.ascontiguousarray(
                    np.concatenate([wq, wk, wv], 0).T
                ).astype(bf16),
                "projT": np.ascontiguousarray(proj_w[:, sl].T).astype(bf16),
                "tril": tril,
            }
        )
    return in_maps


def kernel(x, qkv_w, proj_w, proj_b):
    proj_b = np.asarray(proj_b, dtype=np.float32)

    if "nc" not in _cache:
        _cache["nc"] = _build_nc()
    nc = _cache["nc"]

    in_maps = make_in_maps(x, qkv_w, proj_w)
    res = run_bass_kernel_spmd(nc, in_maps, core_ids=list(range(8)))
    out = np.stack(
        [res.results[2 * b]["out"] + res.results[2 * b + 1]["out"] for b in range(B)], 0
    )
    return (out + proj_b[None, None, :]).astype(np.float32)
